# revision 3
# baseline (speedup 1.0000x reference)
"""MoE top-1 feed-forward (DeepSpeed-style) on 8 Trainium2 NeuronCores.

Strategy (expert parallelism, per the sharding hint):
  - Host computes the (tiny) gate: logits = x @ Wg, softmax, top-1 expert id
    and gate prob per token (float64 for a faithful argmax).
  - Tokens are dispatched to the core owning their expert (core e holds
    W1[e]/b1[e]/W2[e]/b2[e]); each core's token batch is padded to a common
    capacity C so all 8 cores run one SPMD program.
  - Each core runs the dense FFN for its tokens:
        hT = silu(W1^T @ xT + b1);  yT = W2^T @ hT
    with tokens laid out along the free (moving) dimension so no transposes
    are needed on device: xT is [D, C], hT is [F, C], yT is [D, C].
  - Host combines: out[token] = gate * (y + b2[expert]).
"""

import os
import sys

import numpy as np

try:
    import concourse.mybir as mybir  # noqa: F401
except ModuleNotFoundError:  # fallback if the site hooks aren't installed
    sys.path.insert(0, "/opt/trn_rl_repo")

import concourse.mybir as mybir
import concourse.tile as tile
from concourse import bacc
from concourse.bass_utils import run_bass_kernel_spmd

N_CORES = 8

# Compute dtype for the matmuls:
#   "bf16" - weights/activations cast to bfloat16 (f32 PSUM accumulate)
#   "f32r" - fp32 data, PE's replicated-fp32 mode (full rate at N>=256)
#   "f32"  - plain fp32 matmuls (4x slower PE)
MODE = os.environ.get("BASS_MOE_MODE", "f32r")

_CACHE: dict = {}


def _roundup(a: int, m: int) -> int:
    return -(-a // m) * m


def _build_bass(C: int, n_slabs: int, mode: str, D: int, F: int):
    """Build + compile the per-core Bass program for capacity C (divisible by
    n_slabs; slab width CS = C/n_slabs must be 256..512)."""
    f32 = mybir.dt.float32
    if mode == "bf16":
        dt_io = mybir.dt.bfloat16
    elif mode == "f32r":
        dt_io = mybir.dt.float32r
    else:
        dt_io = f32

    KD, KF = D // 128, F // 128
    FG = 4  # f-chunks per W1 group (group covers FG*128 columns of F)
    NG = KF // FG
    CS = C // n_slabs
    assert C % n_slabs == 0 and 1 <= CS <= 512

    def mm(ap):
        return ap

    nc = bacc.Bacc(None, target_bir_lowering=False, debug=False)
    xT = nc.dram_tensor("xT", [D, C], dt_io, kind="ExternalInput")
    w1 = nc.dram_tensor("w1", [D, F], dt_io, kind="ExternalInput")
    w2 = nc.dram_tensor("w2", [F, D], dt_io, kind="ExternalInput")
    b1r = nc.dram_tensor("b1r", [128, KF], f32, kind="ExternalInput")
    yT = nc.dram_tensor("yT", [D, C], f32, kind="ExternalOutput")

    silu = mybir.ActivationFunctionType.Silu

    with tile.TileContext(nc) as tc:
        with (
            tc.tile_pool(name="xp", bufs=1) as xp,
            tc.tile_pool(name="w1p", bufs=2) as w1p,
            tc.tile_pool(name="w2p", bufs=6) as w2p,
            tc.tile_pool(name="hp", bufs=4) as hp,
            tc.tile_pool(name="bp", bufs=1) as bp,
            tc.tile_pool(name="yp", bufs=3) as yp,
            tc.tile_pool(name="ps_h", bufs=2, space="PSUM") as ps_h,
            tc.tile_pool(name="ps_y", bufs=1, space="PSUM") as ps_y,
        ):
            b1t = bp.tile([128, KF], f32, tag="b1", name="b1t")
            nc.sync.dma_start(out=b1t[:], in_=b1r[:])

            for s in range(n_slabs):
                c0 = s * CS
                xt = []
                for d in range(KD):
                    t = xp.tile([128, CS], dt_io, tag=f"x{d}", name=f"xt{d}")
                    nc.sync.dma_start(
                        out=t[:], in_=xT[d * 128 : (d + 1) * 128, c0 : c0 + CS]
                    )
                    xt.append(t)
                py = [
                    ps_y.tile([128, CS], f32, tag=f"y{dd}", name=f"py{dd}")
                    for dd in range(KD)
                ]

                for g in range(NG):
                    w1g = []
                    for d in range(KD):
                        t = w1p.tile([128, FG * 128], dt_io, tag=f"w1_{d}", name=f"w1t{d}")
                        nc.sync.dma_start(
                            out=t[:],
                            in_=w1[
                                d * 128 : (d + 1) * 128,
                                g * FG * 128 : (g + 1) * FG * 128,
                            ],
                        )
                        w1g.append(t)
                    for j in range(FG):
                        f = g * FG + j
                        # hT[f-chunk] = silu(sum_d W1[d, f-chunk]^T @ xT[d] + b1)
                        ph = ps_h.tile([128, CS], f32, tag="hps", name="ph")
                        for d in range(KD):
                            nc.tensor.matmul(
                                ph[:],
                                mm(w1g[d][:, j * 128 : (j + 1) * 128]),
                                mm(xt[d][:]),
                                start=(d == 0),
                                stop=(d == KD - 1),
                            )
                        ht = hp.tile([128, CS], dt_io, tag="ht", name="ht")
                        nc.scalar.activation(ht[:], ph[:], silu, bias=b1t[:, f : f + 1])
                        # yT += W2[f-chunk, :]^T @ hT[f-chunk]
                        w2t = w2p.tile([128, D], dt_io, tag="w2", name="w2t")
                        nc.sync.dma_start(
                            out=w2t[:], in_=w2[f * 128 : (f + 1) * 128, :]
                        )
                        for dd in range(KD):
                            nc.tensor.matmul(
                                py[dd][:],
                                mm(w2t[:, dd * 128 : (dd + 1) * 128]),
                                mm(ht[:]),
                                start=(f == 0),
                                stop=(f == KF - 1),
                            )

                for dd in range(KD):
                    yt = yp.tile([128, CS], f32, tag="yt", name="yt")
                    nc.vector.tensor_copy(yt[:], py[dd][:])
                    nc.sync.dma_start(
                        out=yT[dd * 128 : (dd + 1) * 128, c0 : c0 + CS], in_=yt[:]
                    )

    nc.compile()
    return nc


def _get_bass(C: int, n_slabs: int, mode: str, D: int, F: int):
    key = (C, n_slabs, mode, D, F)
    if key not in _CACHE:
        _CACHE[key] = _build_bass(C, n_slabs, mode, D, F)
    return _CACHE[key]


def _gate_host(x: np.ndarray, Wg: np.ndarray):
    """Top-1 gating in float64: returns (expert_idx [T], gate [T] f32)."""
    logits = x.astype(np.float64) @ Wg.astype(np.float64)
    m = logits.max(-1, keepdims=True)
    p = np.exp(logits - m)
    p /= p.sum(-1, keepdims=True)
    return p.argmax(-1), p.max(-1).astype(np.float32)


def _kernel_numpy(x, Wg, W1, b1, W2, b2):
    """Reference-equivalent fallback (host only)."""
    idx, gate = _gate_host(x, Wg)
    out = np.zeros_like(x)
    for e in range(W1.shape[0]):
        ids = np.nonzero(idx == e)[0]
        if ids.size == 0:
            continue
        h = x[ids] @ W1[e] + b1[e]
        h = h * (1.0 / (1.0 + np.exp(-h)))
        out[ids] = gate[ids, None] * (h @ W2[e] + b2[e])
    return out


def kernel(hidden_states, Wg, W1, b1, W2, b2):
    hidden_states = np.asarray(hidden_states)
    Wg = np.asarray(Wg, dtype=np.float32)
    W1 = np.asarray(W1, dtype=np.float32)
    b1 = np.asarray(b1, dtype=np.float32)
    W2 = np.asarray(W2, dtype=np.float32)
    b2 = np.asarray(b2, dtype=np.float32)

    orig_shape = hidden_states.shape
    D = orig_shape[-1]
    x = np.ascontiguousarray(hidden_states, dtype=np.float32).reshape(-1, D)
    E, _, F = W1.shape

    if E != N_CORES or D % 128 != 0 or F % 128 != 0:
        return _kernel_numpy(x, Wg, W1, b1, W2, b2).reshape(orig_shape)

    idx, gate = _gate_host(x, Wg)
    order = np.argsort(idx, kind="stable")
    counts = np.bincount(idx, minlength=E)
    starts = np.concatenate([[0], np.cumsum(counts)])

    # Capacity: common padded token count per core. Slab width must be
    # 256..512 (PSUM bank limit / fp32r fast path).
    C = max(256, _roundup(int(counts.max()), 32))
    n_slabs = -(-C // 512)
    C = _roundup(C, n_slabs)

    mode = MODE
    np_io = np.float32
    if mode == "bf16":
        import ml_dtypes

        np_io = ml_dtypes.bfloat16

    nc = _get_bass(C, n_slabs, mode, D, F)

    KF = F // 128
    in_maps = []
    for e in range(E):
        ids = order[starts[e] : starts[e + 1]]
        xe = np.zeros((C, D), dtype=np.float32)
        xe[: ids.size] = x[ids]
        in_maps.append(
            {
                "xT": np.ascontiguousarray(xe.T).astype(np_io, copy=False),
                "w1": W1[e].astype(np_io, copy=False),
                "w2": W2[e].astype(np_io, copy=False),
                "b1r": np.ascontiguousarray(b1[e].reshape(KF, 128).T),
            }
        )

    res = run_bass_kernel_spmd(nc, in_maps, list(range(N_CORES)))

    out = np.zeros_like(x)
    for e in range(E):
        ids = order[starts[e] : starts[e + 1]]
        if ids.size == 0:
            continue
        y = res.results[e]["yT"][:, : ids.size].T  # [count, D]
        out[ids] = gate[ids, None] * (y + b2[e])
    return out.reshape(orig_shape)
